# revision 7
# baseline (speedup 1.0000x reference)
"""Builder + host-side prep for nn_Attention distributed kernel.

Strategy: pure data-parallel sharding over (batch, query-row-half).
Core c handles batch b=c//2, query rows i0=(c%2)*512 .. i0+512.
No collectives: K/V are computed per-core from the full sequence of its
batch; each core's output rows are disjoint.

All attention math is in "transposed score" layout S^T[j, i] (j = key
position on partitions, i = query on free axis) so the probability
matrix lands PV-ready without on-chip transposes:
  - q^T, k^T projections: [e, n] layout from x^T (host pre-transposed)
  - S^T  = matmul(lhsT=k^T slice [d,j], rhs=q^T slice [d,i])
  - bias combine alternates per round to balance engines:
      * inject rounds: raw bias^T fp8-e4m3 is ACCUMULATED into the S^T
        PSUM region by a DoubleRow identity matmul (two heads' bias
        tiles share one moving operand; complementary [I|0]/[0|I]
        stationary tiles select one each).  fp8 quantization error
        lands inside the exp argument as a ~0.004 absolute shift.
        P^T = exp(PSUM) in ONE scalar pass, no DVE work.
      * multiply rounds: host-precomputed exp(bias^T) bf16;
        P^T = exp(S^T) * exp(B^T) with the multiply on DVE.
  - out^T_h [33, i] = matmul(lhsT=v_aug [j, 33], rhs=P^T [j, i]) summed
    over j-chunks; column 32 of v_aug is ones -> row 32 = softmax denom.
    Two heads of a pair share one PSUM bank (partition offsets 0 / 64,
    which also lets their PV matmuls co-execute in the PE array).
  - gating sigmoid via tanh (same ACT table set as exp); normalization
    broadcast via DMA-bounce (mid-stream) / indicator matmul (tail);
    gating multiplies + output bias-add run on gpsimd.
"""

import sys

if "/opt/trn_rl_repo" not in sys.path:
    sys.path.insert(0, "/opt/trn_rl_repo")

from contextlib import ExitStack

import ml_dtypes
import numpy as np

import concourse.bass as bass
import concourse.tile as tile
from concourse import bacc, mybir
from concourse.bass import ts

P = 128
B, N, DQ = 4, 1024, 256
H, D = 8, 32
NI = 512  # query rows per core
NCORES = 8

BF16 = mybir.dt.bfloat16
F32 = mybir.dt.float32
F8 = mybir.dt.float8e4
NPBF16 = ml_dtypes.bfloat16
NPF8 = ml_dtypes.float8_e4m3

EXPF = mybir.ActivationFunctionType.Exp
TANHF = mybir.ActivationFunctionType.Tanh
DROW = mybir.MatmulPerfMode.DoubleRow

# rounds whose bias combine runs on the PE (fp8 inject); the rest multiply
# exp(bias) on the DVE.  14+15 included so the kernel tail skips the DVE step.
INJ_ROUNDS = frozenset({0, 2, 4, 6, 8, 10, 12, 14, 15})


def build_nc():
    nc = bacc.Bacc(None, target_bir_lowering=False, debug=False)

    # DRAM parameters (identical graph on all 8 cores; shards differ)
    # raw bias^T fp8 for inject rounds, paired by head-pair:
    # eb[hp, jp, p, kt, t*512+i] = bias^T[2*hp+kt][(jp*2+t)*128+p, i]
    eb_d = nc.declare_dram_parameter("eb", [4, 4, P, 2, 2 * NI], F8, False)
    # exp(bias^T) bf16 for multiply rounds, per head:
    # ebx[hp, jp, u, p, t, i] = exp(bias^T)[2*hp+u][(jp*2+t)*128+p, i]
    ebx_d = nc.declare_dram_parameter("ebx", [4, 4, 2, P, 2, NI], BF16, False)
    # bf16 weights + x packed per-partition, ordered so the earliest DMAs
    # carry what the first projections need:
    # [wq 512 | xqt 1024 | wk 512 | xt 2048 | wv 512 | wg 512 | wo 512]
    wpack_d = nc.declare_dram_parameter("wpack", [P, 5632], BF16, False)
    # [I|0] / [0|I] fp8 stationary tiles for the DoubleRow bias inject
    id_d = nc.declare_dram_parameter("idm", [P, 2, 2 * P], F8, False)
    hbg_d = nc.declare_dram_parameter("hbg", [P, 2], F32, False)   # bg/2 as [p, chunk]
    bob_d = nc.declare_dram_parameter("bob", [P, 2], F32, False)   # bo as [p, c_chunk]
    ind_d = nc.declare_dram_parameter("ind", [8, 256], BF16, False)  # (e//32 == h)
    y_d = nc.declare_dram_parameter("out", [2, P, NI], F32, True)  # y^T chunked
    dnrec_d = nc.dram_tensor("dnrec", [8, NI], F32)                # internal scratch

    from concourse.tile_rust import add_dep_helper

    with tile.TileContext(nc) as tc, ExitStack() as ctx:
        singles = ctx.enter_context(tc.tile_pool(name="singles", bufs=1))
        spsum = ctx.enter_context(tc.tile_pool(name="spsum", bufs=3, space="PSUM"))
        opsum = ctx.enter_context(tc.tile_pool(name="opsum", bufs=2, space="PSUM"))
        ebuf = ctx.enter_context(tc.tile_pool(name="ebuf", bufs=4))
        esb = ctx.enter_context(tc.tile_pool(name="esb", bufs=3))
        ptb = ctx.enter_context(tc.tile_pool(name="ptb", bufs=3))
        ostb = ctx.enter_context(tc.tile_pool(name="ostb", bufs=2))

        _sc = [0]

        def sslot():
            _sc[0] += 1
            return spsum.tile([P, 1024], F32, tag="s_ps", name=f"s_ps{_sc[0]}")

        # ---- load constants / weights (3 prioritized DMAs) ----
        wpack_sb = singles.tile([P, 5632], BF16)
        nc.sync.dma_start(out=wpack_sb[:, 0:1536], in_=wpack_d[:, 0:1536])
        id_sb = singles.tile([P, 2, 2 * P], F8)
        nc.sync.dma_start(out=id_sb, in_=id_d[:])
        hbg_sb = singles.tile([P, 2], F32)
        bob_sb = singles.tile([P, 2], F32)
        ind_sb = singles.tile([8, 256], BF16)
        nc.sync.dma_start(out=hbg_sb, in_=hbg_d[:])
        nc.sync.dma_start(out=bob_sb, in_=bob_d[:])
        nc.sync.dma_start(out=ind_sb, in_=ind_d[:])
        nc.sync.dma_start(out=wpack_sb[:, 1536:4096], in_=wpack_d[:, 1536:4096])
        # v/g/o weights on the gpsimd queue, concurrent with the above
        nc.gpsimd.dma_start(out=wpack_sb[:, 4096:5632], in_=wpack_d[:, 4096:5632])

        wq_sb = wpack_sb[:, 0:512].rearrange("p (k e) -> p k e", k=2)
        xqt_sb = wpack_sb[:, 512:1536].rearrange("p (k i) -> p k i", k=2)
        wk_sb = wpack_sb[:, 1536:2048].rearrange("p (k e) -> p k e", k=2)
        xt_sb = wpack_sb[:, 2048:4096].rearrange("p (k n) -> p k n", k=2)
        wv_sb = wpack_sb[:, 4096:4608].rearrange("p (k e) -> p k e", k=2)
        wg_sb = wpack_sb[:, 4608:5120].rearrange("p (k e) -> p k e", k=2)
        wo_sb = wpack_sb[:, 5120:5632].rearrange("p (k e) -> p k e", k=2)

        # ACT table preload: dummy Exp at t=0 so the table load overlaps
        # the weight DMA instead of stalling the first real use
        warm = singles.tile([P, 8], F32)
        nc.vector.memset(warm, 1.0)
        nc.scalar.activation(out=warm, in_=warm, func=EXPF)

        # ---- projection targets ----
        kt_sb = [singles.tile([P, N], BF16, name=f"kt{m}") for m in range(2)]
        qt_sb = [singles.tile([P, NI], BF16, name=f"qt{m}") for m in range(2)]
        vaug_sb = [singles.tile([P, 2, H, 33], BF16, name=f"vaug{j}")
                   for j in range(4)]
        sig_sb = singles.tile([P, 2, NI], F32)    # sigmoid(gates)^T [e, i]
        ogt_un = singles.tile([P, 2, NI], F32)    # unnormalized gated^T staging

        def v_round(jtp):
            ps = sslot()
            f = l = None
            for u in range(2):
                jt = jtp * 2 + u
                for kc in range(2):
                    l = nc.tensor.matmul(
                        ps[:, u * 512 : u * 512 + 256],
                        lhsT=xt_sb[:, kc, ts(jt, P)], rhs=wv_sb[:, kc, :],
                        start=(kc == 0), stop=(kc == 1),
                    )
                    f = f or l
            for u in range(2):
                nc.vector.tensor_copy(
                    out=vaug_sb[jtp][:, u, :, 0:32],
                    in_=ps[:, u * 512 : u * 512 + 256].rearrange(
                        "p (h d) -> p h d", h=H),
                )
            return f, l

        def qk_round(m, part):
            # part 0: q chunk m + k chunk m first half; part 1: k second half
            ps = sslot()
            f = l = None
            if part == 0:
                for kc in range(2):
                    l = nc.tensor.matmul(
                        ps[:, :NI], lhsT=wq_sb[:, kc, ts(m, P)],
                        rhs=xqt_sb[:, kc, :], start=(kc == 0), stop=(kc == 1))
                    f = f or l
                for kc in range(2):
                    l = nc.tensor.matmul(
                        ps[:, NI:], lhsT=wk_sb[:, kc, ts(m, P)],
                        rhs=xt_sb[:, kc, :512], start=(kc == 0), stop=(kc == 1))
                nc.vector.tensor_copy(out=qt_sb[m], in_=ps[:, :NI])
                nc.vector.tensor_copy(out=kt_sb[m][:, 0:512], in_=ps[:, NI:])
            else:
                for kc in range(2):
                    l = nc.tensor.matmul(
                        ps[:, :NI], lhsT=wk_sb[:, kc, ts(m, P)],
                        rhs=xt_sb[:, kc, 512:], start=(kc == 0), stop=(kc == 1))
                    f = f or l
                nc.vector.tensor_copy(out=kt_sb[m][:, 512:], in_=ps[:, :NI])
            return f, l

        def g_round():
            # gates^T: sigmoid via tanh: sig = 0.5*tanh((g+bg)/2) + 0.5
            ps = sslot()
            f = l = None
            for m in range(2):
                for kc in range(2):
                    l = nc.tensor.matmul(
                        ps[:, ts(m, NI)], lhsT=wg_sb[:, kc, ts(m, P)],
                        rhs=xqt_sb[:, kc, :], start=(kc == 0), stop=(kc == 1))
                    f = f or l
            for m in range(2):
                nc.scalar.activation(out=sig_sb[:, m, :], in_=ps[:, ts(m, NI)],
                                     func=TANHF, bias=hbg_sb[:, m : m + 1],
                                     scale=0.5)
            nc.vector.tensor_scalar(out=sig_sb, in0=sig_sb, scalar1=0.5,
                                    scalar2=0.5, op0=mybir.AluOpType.mult,
                                    op1=mybir.AluOpType.add)
            return f, l

        # pre-stream: q/k chunk 0 + first v pair
        qk_round(0, 0)
        qk_round(0, 1)
        v_round(0)
        for j in range(4):
            nc.vector.memset(vaug_sb[j][:, :, :, 32:33], 1.0)

        # injected projection work, keyed by stream round
        inject = {
            1: [lambda: v_round(1)],
            2: [lambda: v_round(2)],
            3: [lambda: v_round(3)],
            4: [lambda: qk_round(1, 0)],
            5: [lambda: qk_round(1, 1)],
            6: [g_round],
        }

        # ---- attention stream state ----
        dn8 = singles.tile([8, NI], F32)        # per-head denominators
        nc.vector.memset(dn8, 1.0)              # rows read before all written
        sigf = singles.tile([P, 2, NI], F32)    # sig * (1/denom broadcast)
        ogt = singles.tile([P, 2, NI], BF16)    # normalized gated out^T

        qk_insts, pv_insts, inj_insts = [], [], []

        def chunk_tail(hc, dmae):
            # reciprocal of denominators; broadcast each head's row to its 32
            # partitions; gating multiplies on gpsimd (SBUF-only)
            rec = ostb.tile([8, NI], F32, tag="rec", name=f"rec{hc}")
            nc.vector.reciprocal_approx_fast(out=rec, in_=dn8)
            if dmae is None:
                # end of stream: PE is idle -> indicator-matmul broadcast
                recb = ostb.tile([8, NI], BF16, tag="recb", name=f"recb{hc}")
                nc.vector.tensor_copy(out=recb, in_=rec)
                bps = opsum.tile([P, NI], F32, tag="o", name=f"bps{hc}")
                nc.tensor.matmul(bps[:, :NI], lhsT=ind_sb[:, ts(hc, P)],
                                 rhs=recb, start=True, stop=True)
                nc.vector.tensor_mul(out=sigf[:, hc, :], in0=sig_sb[:, hc, :],
                                     in1=bps[:, :NI])
                nc.gpsimd.tensor_mul(out=ogt[:, hc, :], in0=ogt_un[:, hc, :],
                                     in1=sigf[:, hc, :])
                return
            # mid-stream: DRAM bounce with a stride-0 AP (no compute engine
            # in the path -> no PE queue blocking)
            dmae.dma_start(out=dnrec_d[:], in_=rec)
            rb = ostb.tile([P, NI], F32, tag="rb", name=f"rb{hc}")
            for g in range(4):
                sl = dnrec_d[hc * 4 + g : hc * 4 + g + 1, :]
                bcast_ap = bass.AP(tensor=sl.tensor, offset=sl.offset,
                                   ap=[[0, 32], list(sl.ap[1])])
                dmae.dma_start(out=rb[g * 32 : (g + 1) * 32, :], in_=bcast_ap)
            nc.gpsimd.tensor_mul(out=sigf[:, hc, :], in0=sig_sb[:, hc, :],
                                 in1=rb)
            nc.gpsimd.tensor_mul(out=ogt[:, hc, :], in0=ogt_un[:, hc, :],
                                 in1=sigf[:, hc, :])

        o_tiles = {}
        pt_tiles = {}

        def emit_pv(r):
            hp, jp = divmod(r, 4)
            if hp not in o_tiles:
                o_tiles[hp] = opsum.tile([P, NI], F32, tag="o", name=f"o{hp}")
            o_t = o_tiles[hp]
            pt = pt_tiles.pop(r)
            f = l = None
            for t in range(2):
                jc = jp * 2 + t
                for u, h in enumerate((2 * hp, 2 * hp + 1)):
                    l = nc.tensor.matmul(
                        o_t[64 * u : 64 * u + 33, :],
                        lhsT=vaug_sb[jc // 2][:, jc % 2, h, :],
                        rhs=pt[h][:, t, :],
                        start=(jc == 0), stop=(jc == 7),
                        skip_group_check=True,
                    )
                    f = f or l
            return f, l

        def emit_dumps(hp):
            # evacuate PSUM on DVE (gpsimd can't read PSUM), then
            # cross-partition DMA moves on the gpsimd SWDGE queues
            o_t = o_tiles.pop(hp)
            ost = ostb.tile([P, NI], F32, tag="ost", name=f"ost{hp}")
            for u in range(2):
                nc.vector.tensor_copy(out=ost[64 * u : 64 * u + 33, :],
                                      in_=o_t[64 * u : 64 * u + 33, :])
            for u, h in enumerate((2 * hp, 2 * hp + 1)):
                hq, hc = h % 4, h // 4
                nc.gpsimd.dma_start(
                    out=ogt_un[hq * 32 : (hq + 1) * 32, hc, :],
                    in_=ost[64 * u : 64 * u + 32, :],
                )
                nc.gpsimd.dma_start(out=dn8[h : h + 1, :],
                                    in_=ost[64 * u + 32 : 64 * u + 33, :])

        for r in range(16):
            hp, jp = divmod(r, 4)
            heads = (2 * hp, 2 * hp + 1)
            use_inj = r in INJ_ROUNDS
            if use_inj:
                ebt = ebuf.tile([P, 2, 2 * NI], F8, tag="ebt", name=f"ebt{r}")
                nc.sync.dma_start(out=ebt, in_=eb_d[hp, jp])
            else:
                ebx = {}
                for u, h in enumerate(heads):
                    ebx[h] = ebuf.tile([P, 2, NI], BF16, tag="ebx",
                                       name=f"ebx{r}_{u}")
                    nc.sync.dma_start(out=ebx[h], in_=ebx_d[hp, jp, u])
            s_ps = {h: sslot() for h in heads}
            # QK batch: self-contained groups so the two heads' matmuls
            # co-execute in the array (different 32-row tile groups)
            qf = ql = None
            for t in range(2):
                jc = jp * 2 + t
                for h in heads:
                    hq, hc = h % 4, h // 4
                    ql = nc.tensor.matmul(
                        s_ps[h][:, ts(t, 512)],
                        lhsT=kt_sb[hc][hq * 32 : (hq + 1) * 32, ts(jc, P)],
                        rhs=qt_sb[hc][hq * 32 : (hq + 1) * 32, :],
                        start=True, stop=True,
                        tile_position=(hq * 32, 0),
                        skip_group_check=True,
                    )
                    qf = qf or ql
            last = ql
            if use_inj:
                # bias inject: accumulates B^T on top of each t-subregion
                # (matmul out can't cross a PSUM bank boundary)
                for u, h in enumerate(heads):
                    for t in range(2):
                        last = nc.tensor.matmul(
                            s_ps[h][:, ts(t, 512)],
                            lhsT=id_sb[:, :, ts(u, P)],
                            rhs=ebt[:, :, ts(t, 512)],
                            start=False, stop=True,
                            perf_mode=DROW,
                            skip_group_check=True,
                        )
                        add_dep_helper(last.ins, ql.ins, sync=False,
                                       reason="bias inject after qk batch")
            qk_insts.append((qf, last))
            # injected projection work between the QK and PV batches
            ij = []
            for fn in inject.get(r, []):
                ij.append(fn())
            inj_insts.append(ij)
            # exp per head: PSUM -> bf16 P^T in SBUF (multiply rounds apply
            # exp(bias) on the DVE afterwards)
            pt = {}
            for h in heads:
                pt[h] = ptb.tile([P, 2, NI], BF16, tag="pt", name=f"pt{h}_{jp}")
                if use_inj:
                    nc.scalar.activation(
                        out=pt[h],
                        in_=s_ps[h][:].rearrange("p (t i) -> p t i", t=2),
                        func=EXPF,
                    )
                else:
                    es = esb.tile([P, 2, NI], BF16, tag="es",
                                  name=f"es{h}_{jp}")
                    nc.scalar.activation(
                        out=es,
                        in_=s_ps[h][:].rearrange("p (t i) -> p t i", t=2),
                        func=EXPF,
                    )
                    nc.vector.tensor_mul(out=pt[h], in0=es, in1=ebx[h])
            pt_tiles[r] = pt
            if r >= 1:
                pv_insts.append(emit_pv(r - 1))
            if jp == 0 and hp >= 1:
                emit_dumps(hp - 1)
            if r == 9:
                chunk_tail(0, nc.gpsimd)
        pv_insts.append(emit_pv(15))
        emit_dumps(3)

        # PE batch order: [QK+inj][proj][PV] per round, staggered
        for r in range(len(qk_insts)):
            if r >= 2:
                add_dep_helper(qk_insts[r][0].ins, pv_insts[r - 2][1].ins,
                               sync=False, reason="qk(r) after pv(r-2)")
            for f, l in inj_insts[r]:
                add_dep_helper(f.ins, qk_insts[r][1].ins, sync=False,
                               reason="proj after qk batch")
            if r + 1 < len(qk_insts):
                prev = (inj_insts[r + 1][-1][1] if inj_insts[r + 1]
                        else qk_insts[r + 1][1])
                add_dep_helper(pv_insts[r][0].ins, prev.ins, sync=False,
                               reason="pv(r) after qk/inj(r+1)")

        chunk_tail(1, None)

        # ---- tail: output projection y^T[c, i] ----
        for cc in range(2):
            yps = sslot()
            for ec in range(2):
                nc.tensor.matmul(
                    yps[:, :NI], lhsT=wo_sb[:, ec, ts(cc, P)], rhs=ogt[:, ec, :],
                    start=(ec == 0), stop=(ec == 1),
                )
            ysb = ostb.tile([P, NI], F32, tag="ysb")
            nc.vector.tensor_scalar_add(out=ysb, in0=yps[:, :NI],
                                        scalar1=bob_sb[:, cc : cc + 1])
            nc.sync.dma_start(out=y_d[cc], in_=ysb)

    nc.compile()
    return nc


def prep_core_inputs(core, x, mask, attn_bias, Wq, Wkv, Wo, bo, Wg, bg):
    """Host-side shard + layout prep for one core. All numpy."""
    b, ih = core // 2, core % 2
    i0 = ih * NI
    scale = D ** -0.5

    xt = np.ascontiguousarray(x[b].T)  # [256, N]
    amask = np.where(mask[b] > 0, 0.0, -200.0).astype(np.float32)  # [N] over j
    bt = attn_bias[b, :, i0 : i0 + NI, :].transpose(0, 2, 1)  # [H, j, i]
    bt = bt + amask[None, :, None]
    bt6 = bt.reshape(4, 2, 4, 2, P, NI)  # [hp, u, jp, t, p, i]
    # inject rounds: raw fp8, paired:
    # eb[hp, jp, p, kt, t*512+i] = bt[2*hp+kt, (jp*2+t)*128+p, i]
    eb = (
        bt6.transpose(0, 2, 4, 1, 3, 5).reshape(4, 4, P, 2, 2 * NI)
    ).astype(NPF8)
    # multiply rounds: exp(bias) bf16 per head:
    ebx = np.exp(bt6.transpose(0, 2, 1, 4, 3, 5)).astype(NPBF16)  # [hp,jp,u,p,t,i]

    def chunk(wT):  # [256, X] -> [2, 128, X] bf16
        return np.ascontiguousarray(wT.reshape(2, P, -1)).astype(NPBF16)

    ind = np.zeros((8, 256), np.float32)
    for h in range(H):
        ind[h, h * 32 : (h + 1) * 32] = 1.0

    idm = np.zeros((P, 2, 2 * P), np.float32)
    for u in range(2):
        idm[:, u, u * P : (u + 1) * P] = np.eye(P)

    def flat(wT):  # [256, X] -> [128, 2*X] per-partition pack
        c = chunk(wT)  # [2, 128, X]
        return c.transpose(1, 0, 2).reshape(P, -1)

    wpack = np.concatenate(
        [flat((Wq * scale).T), flat(xt[:, i0 : i0 + NI]), flat(Wkv[:256].T),
         flat(xt), flat(Wkv[256:].T), flat(Wg.T), flat(Wo.T)], axis=1)
    return {
        "wpack": np.ascontiguousarray(wpack),
        "eb": np.ascontiguousarray(eb),
        "ebx": np.ascontiguousarray(ebx),
        "idm": idm.astype(NPF8),
        "hbg": np.ascontiguousarray((bg * 0.5).reshape(2, P).T).astype(np.float32),
        "bob": np.ascontiguousarray(bo.astype(np.float32).reshape(2, P).T),
        "ind": ind.astype(NPBF16),
    }


def prep_all_inputs(**inputs):
    inputs = {k: np.asarray(v, dtype=np.float32) for k, v in inputs.items()}
    return [prep_core_inputs(c, **inputs) for c in range(NCORES)]


def gather_outputs(results):
    """results: per-core dicts with 'out' = y^T chunked [2, P, NI] -> [B, N, DQ]."""
    y = np.zeros((B, N, DQ), np.float32)
    for c in range(NCORES):
        b, ih = c // 2, c % 2
        yt = np.asarray(results[c]["out"]).reshape(DQ, NI)  # [c, i]
        y[b, ih * NI : (ih + 1) * NI, :] = yt.T
    return y


_NC_CACHE = None


def _get_nc():
    global _NC_CACHE
    if _NC_CACHE is None:
        _NC_CACHE = build_nc()
    return _NC_CACHE


def kernel(**inputs):
    """Full (unsharded) inputs -> full [B, N, DQ] output, on 8 NeuronCores."""
    from concourse.bass_utils import run_bass_kernel_spmd

    nc = _get_nc()
    in_maps = prep_all_inputs(**inputs)
    res = run_bass_kernel_spmd(nc, in_maps, list(range(NCORES)))
    return gather_outputs(res.results)


# revision 8
# speedup vs baseline: 1.1160x; 1.1160x over previous
"""Builder + host-side prep for nn_Attention distributed kernel.

Strategy: pure data-parallel sharding over (batch, query-row-half).
Core c handles batch b=c//2, query rows i0=(c%2)*512 .. i0+512.
No collectives: K/V are computed per-core from the full sequence of its
batch; each core's output rows are disjoint.

All attention math is in "transposed score" layout S^T[j, i] (j = key
position on partitions, i = query on free axis) so the probability
matrix lands PV-ready without on-chip transposes:
  - q^T, k^T projections: [e, n] layout from x^T (host pre-transposed)
  - S^T  = matmul(lhsT=k^T slice [d,j], rhs=q^T slice [d,i])
  - bias combine alternates per round to balance engines:
      * inject rounds: raw bias^T fp8-e4m3 is ACCUMULATED into the S^T
        PSUM region by a DoubleRow identity matmul (two heads' bias
        tiles share one moving operand; complementary [I|0]/[0|I]
        stationary tiles select one each).  fp8 quantization error
        lands inside the exp argument as a ~0.004 absolute shift.
        P^T = exp(PSUM) in ONE scalar pass, no DVE work.
      * multiply rounds: host-precomputed exp(bias^T) bf16;
        P^T = exp(S^T) * exp(B^T) with the multiply on DVE.
  - out^T_h [33, i] = matmul(lhsT=v_aug [j, 33], rhs=P^T [j, i]) summed
    over j-chunks; column 32 of v_aug is ones -> row 32 = softmax denom.
    Two heads of a pair share one PSUM bank (partition offsets 0 / 64,
    which also lets their PV matmuls co-execute in the PE array).
  - gating sigmoid via tanh (same ACT table set as exp); normalization
    broadcast via DMA-bounce (mid-stream) / indicator matmul (tail);
    gating multiplies + output bias-add run on gpsimd.
"""

import sys

if "/opt/trn_rl_repo" not in sys.path:
    sys.path.insert(0, "/opt/trn_rl_repo")

from contextlib import ExitStack

import ml_dtypes
import numpy as np

import concourse.bass as bass
import concourse.tile as tile
from concourse import bacc, mybir
from concourse.bass import ts

P = 128
B, N, DQ = 4, 1024, 256
H, D = 8, 32
NI = 512  # query rows per core
NCORES = 8

BF16 = mybir.dt.bfloat16
F32 = mybir.dt.float32
F8 = mybir.dt.float8e4
NPBF16 = ml_dtypes.bfloat16
NPF8 = ml_dtypes.float8_e4m3

EXPF = mybir.ActivationFunctionType.Exp
TANHF = mybir.ActivationFunctionType.Tanh
DROW = mybir.MatmulPerfMode.DoubleRow

# rounds whose bias combine runs on the PE (fp8 inject); the rest multiply
# exp(bias) on the DVE.  14+15 included so the kernel tail skips the DVE step.
INJ_ROUNDS = frozenset(range(16))


def build_nc():
    nc = bacc.Bacc(None, target_bir_lowering=False, debug=False)

    # DRAM parameters (identical graph on all 8 cores; shards differ)
    # raw bias^T fp8 for inject rounds, paired by head-pair:
    # eb[hp, jp, p, kt, t*512+i] = bias^T[2*hp+kt][(jp*2+t)*128+p, i]
    eb_d = nc.declare_dram_parameter("eb", [4, 4, P, 2, 2 * NI], F8, False)
    # exp(bias^T) bf16 for multiply rounds, per head:
    # ebx[hp, jp, u, p, t, i] = exp(bias^T)[2*hp+u][(jp*2+t)*128+p, i]
    ebx_d = nc.declare_dram_parameter("ebx", [4, 4, 2, P, 2, NI], BF16, False)
    # bf16 weights + x packed per-partition, ordered so the earliest DMAs
    # carry what the first projections need:
    # [wq 512 | xqt 1024 | wk 512 | xt 2048 | wv 512 | wg 512 | wo 512]
    wpack_d = nc.declare_dram_parameter("wpack", [P, 5632], BF16, False)
    # [I|0] / [0|I] fp8 stationary tiles for the DoubleRow bias inject
    id_d = nc.declare_dram_parameter("idm", [P, 2, 2 * P], F8, False)
    hbg_d = nc.declare_dram_parameter("hbg", [P, 2], F32, False)   # bg/2 as [p, chunk]
    bob_d = nc.declare_dram_parameter("bob", [P, 2], F32, False)   # bo as [p, c_chunk]
    ind_d = nc.declare_dram_parameter("ind", [8, 256], BF16, False)  # (e//32 == h)
    y_d = nc.declare_dram_parameter("out", [2, P, NI], F32, True)  # y^T chunked
    dnrec_d = nc.dram_tensor("dnrec", [8, NI], F32)                # internal scratch

    from concourse.tile_rust import add_dep_helper

    with tile.TileContext(nc) as tc, ExitStack() as ctx:
        singles = ctx.enter_context(tc.tile_pool(name="singles", bufs=1))
        spsum = ctx.enter_context(tc.tile_pool(name="spsum", bufs=3, space="PSUM"))
        opsum = ctx.enter_context(tc.tile_pool(name="opsum", bufs=2, space="PSUM"))
        ebuf = ctx.enter_context(tc.tile_pool(name="ebuf", bufs=4))
        esb = ctx.enter_context(tc.tile_pool(name="esb", bufs=3))
        ptb = ctx.enter_context(tc.tile_pool(name="ptb", bufs=3))
        ostb = ctx.enter_context(tc.tile_pool(name="ostb", bufs=2))

        _sc = [0]

        def sslot():
            _sc[0] += 1
            return spsum.tile([P, 1024], F32, tag="s_ps", name=f"s_ps{_sc[0]}")

        # ---- load constants / weights (3 prioritized DMAs) ----
        wpack_sb = singles.tile([P, 5632], BF16)
        nc.sync.dma_start(out=wpack_sb[:, 0:1536], in_=wpack_d[:, 0:1536])
        id_sb = singles.tile([P, 2, 2 * P], F8)
        nc.sync.dma_start(out=id_sb, in_=id_d[:])
        hbg_sb = singles.tile([P, 2], F32)
        bob_sb = singles.tile([P, 2], F32)
        ind_sb = singles.tile([8, 256], BF16)
        nc.sync.dma_start(out=hbg_sb, in_=hbg_d[:])
        nc.sync.dma_start(out=bob_sb, in_=bob_d[:])
        nc.sync.dma_start(out=ind_sb, in_=ind_d[:])
        nc.sync.dma_start(out=wpack_sb[:, 1536:4096], in_=wpack_d[:, 1536:4096])
        # v/g/o weights on the gpsimd queue, concurrent with the above
        nc.gpsimd.dma_start(out=wpack_sb[:, 4096:5632], in_=wpack_d[:, 4096:5632])

        wq_sb = wpack_sb[:, 0:512].rearrange("p (k e) -> p k e", k=2)
        xqt_sb = wpack_sb[:, 512:1536].rearrange("p (k i) -> p k i", k=2)
        wk_sb = wpack_sb[:, 1536:2048].rearrange("p (k e) -> p k e", k=2)
        xt_sb = wpack_sb[:, 2048:4096].rearrange("p (k n) -> p k n", k=2)
        wv_sb = wpack_sb[:, 4096:4608].rearrange("p (k e) -> p k e", k=2)
        wg_sb = wpack_sb[:, 4608:5120].rearrange("p (k e) -> p k e", k=2)
        wo_sb = wpack_sb[:, 5120:5632].rearrange("p (k e) -> p k e", k=2)

        # ACT table preload: dummy Exp at t=0 so the table load overlaps
        # the weight DMA instead of stalling the first real use
        warm = singles.tile([P, 8], F32)
        nc.vector.memset(warm, 1.0)
        nc.scalar.activation(out=warm, in_=warm, func=EXPF)

        # ---- projection targets ----
        kt_sb = [singles.tile([P, N], BF16, name=f"kt{m}") for m in range(2)]
        qt_sb = [singles.tile([P, NI], BF16, name=f"qt{m}") for m in range(2)]
        vaug_sb = [singles.tile([P, 2, H, 33], BF16, name=f"vaug{j}")
                   for j in range(4)]
        sig_sb = singles.tile([P, 2, NI], F32)    # sigmoid(gates)^T [e, i]
        ogt_un = singles.tile([P, 2, NI], F32)    # unnormalized gated^T staging

        def v_round(jtp):
            ps = sslot()
            f = l = None
            for u in range(2):
                jt = jtp * 2 + u
                for kc in range(2):
                    l = nc.tensor.matmul(
                        ps[:, u * 512 : u * 512 + 256],
                        lhsT=xt_sb[:, kc, ts(jt, P)], rhs=wv_sb[:, kc, :],
                        start=(kc == 0), stop=(kc == 1),
                    )
                    f = f or l
            for u in range(2):
                nc.vector.tensor_copy(
                    out=vaug_sb[jtp][:, u, :, 0:32],
                    in_=ps[:, u * 512 : u * 512 + 256].rearrange(
                        "p (h d) -> p h d", h=H),
                )
            return f, l

        def qk_round(m, part):
            # part 0: q chunk m + k chunk m first half; part 1: k second half
            ps = sslot()
            f = l = None
            if part == 0:
                for kc in range(2):
                    l = nc.tensor.matmul(
                        ps[:, :NI], lhsT=wq_sb[:, kc, ts(m, P)],
                        rhs=xqt_sb[:, kc, :], start=(kc == 0), stop=(kc == 1))
                    f = f or l
                for kc in range(2):
                    l = nc.tensor.matmul(
                        ps[:, NI:], lhsT=wk_sb[:, kc, ts(m, P)],
                        rhs=xt_sb[:, kc, :512], start=(kc == 0), stop=(kc == 1))
                nc.vector.tensor_copy(out=qt_sb[m], in_=ps[:, :NI])
                nc.vector.tensor_copy(out=kt_sb[m][:, 0:512], in_=ps[:, NI:])
            else:
                for kc in range(2):
                    l = nc.tensor.matmul(
                        ps[:, :NI], lhsT=wk_sb[:, kc, ts(m, P)],
                        rhs=xt_sb[:, kc, 512:], start=(kc == 0), stop=(kc == 1))
                    f = f or l
                nc.vector.tensor_copy(out=kt_sb[m][:, 512:], in_=ps[:, :NI])
            return f, l

        def g_round():
            # gates^T: sigmoid via tanh: sig = 0.5*tanh((g+bg)/2) + 0.5
            ps = sslot()
            f = l = None
            for m in range(2):
                for kc in range(2):
                    l = nc.tensor.matmul(
                        ps[:, ts(m, NI)], lhsT=wg_sb[:, kc, ts(m, P)],
                        rhs=xqt_sb[:, kc, :], start=(kc == 0), stop=(kc == 1))
                    f = f or l
            for m in range(2):
                nc.scalar.activation(out=sig_sb[:, m, :], in_=ps[:, ts(m, NI)],
                                     func=TANHF, bias=hbg_sb[:, m : m + 1],
                                     scale=0.5)
            nc.vector.tensor_scalar(out=sig_sb, in0=sig_sb, scalar1=0.5,
                                    scalar2=0.5, op0=mybir.AluOpType.mult,
                                    op1=mybir.AluOpType.add)
            return f, l

        # pre-stream: q/k chunk 0 + first v pair
        qk_round(0, 0)
        qk_round(0, 1)
        v_round(0)
        for j in range(4):
            nc.vector.memset(vaug_sb[j][:, :, :, 32:33], 1.0)

        # injected projection work, keyed by stream round
        inject = {
            1: [lambda: v_round(1)],
            2: [lambda: v_round(2)],
            3: [lambda: v_round(3)],
            4: [lambda: qk_round(1, 0)],
            5: [lambda: qk_round(1, 1)],
            6: [g_round],
        }

        # ---- attention stream state ----
        dn8 = singles.tile([8, NI], F32)        # per-head denominators
        nc.vector.memset(dn8, 1.0)              # rows read before all written
        sigf = singles.tile([P, 2, NI], F32)    # sig * (1/denom broadcast)
        ogt = singles.tile([P, 2, NI], BF16)    # normalized gated out^T

        qk_insts, pv_insts, inj_insts = [], [], []

        def chunk_tail(hc, dmae):
            # reciprocal of denominators; broadcast each head's row to its 32
            # partitions; gating multiplies on gpsimd (SBUF-only)
            rec = ostb.tile([8, NI], F32, tag="rec", name=f"rec{hc}")
            nc.vector.reciprocal_approx_fast(out=rec, in_=dn8)
            if dmae is None:
                # end of stream: PE is idle -> indicator-matmul broadcast
                recb = ostb.tile([8, NI], BF16, tag="recb", name=f"recb{hc}")
                nc.vector.tensor_copy(out=recb, in_=rec)
                bps = opsum.tile([P, NI], F32, tag="o", name=f"bps{hc}")
                nc.tensor.matmul(bps[:, :NI], lhsT=ind_sb[:, ts(hc, P)],
                                 rhs=recb, start=True, stop=True)
                nc.vector.tensor_mul(out=sigf[:, hc, :], in0=sig_sb[:, hc, :],
                                     in1=bps[:, :NI])
                nc.gpsimd.tensor_mul(out=ogt[:, hc, :], in0=ogt_un[:, hc, :],
                                     in1=sigf[:, hc, :])
                return
            # mid-stream: DRAM bounce with a stride-0 AP (no compute engine
            # in the path -> no PE queue blocking)
            dmae.dma_start(out=dnrec_d[:], in_=rec)
            rb = ostb.tile([P, NI], F32, tag="rb", name=f"rb{hc}")
            for g in range(4):
                sl = dnrec_d[hc * 4 + g : hc * 4 + g + 1, :]
                bcast_ap = bass.AP(tensor=sl.tensor, offset=sl.offset,
                                   ap=[[0, 32], list(sl.ap[1])])
                dmae.dma_start(out=rb[g * 32 : (g + 1) * 32, :], in_=bcast_ap)
            nc.gpsimd.tensor_mul(out=sigf[:, hc, :], in0=sig_sb[:, hc, :],
                                 in1=rb)
            nc.gpsimd.tensor_mul(out=ogt[:, hc, :], in0=ogt_un[:, hc, :],
                                 in1=sigf[:, hc, :])

        o_tiles = {}
        pt_tiles = {}

        def emit_pv(r):
            hp, jp = divmod(r, 4)
            if hp not in o_tiles:
                o_tiles[hp] = opsum.tile([P, NI], F32, tag="o", name=f"o{hp}")
            o_t = o_tiles[hp]
            pt = pt_tiles.pop(r)
            f = l = None
            for t in range(2):
                jc = jp * 2 + t
                for u, h in enumerate((2 * hp, 2 * hp + 1)):
                    l = nc.tensor.matmul(
                        o_t[64 * u : 64 * u + 33, :],
                        lhsT=vaug_sb[jc // 2][:, jc % 2, h, :],
                        rhs=pt[h][:, t, :],
                        start=(jc == 0), stop=(jc == 7),
                        skip_group_check=True,
                    )
                    f = f or l
            return f, l

        def emit_dumps(hp):
            # evacuate PSUM on DVE (gpsimd can't read PSUM), then
            # cross-partition DMA moves on the gpsimd SWDGE queues
            o_t = o_tiles.pop(hp)
            ost = ostb.tile([P, NI], F32, tag="ost", name=f"ost{hp}")
            for u in range(2):
                nc.vector.tensor_copy(out=ost[64 * u : 64 * u + 33, :],
                                      in_=o_t[64 * u : 64 * u + 33, :])
            for u, h in enumerate((2 * hp, 2 * hp + 1)):
                hq, hc = h % 4, h // 4
                nc.gpsimd.dma_start(
                    out=ogt_un[hq * 32 : (hq + 1) * 32, hc, :],
                    in_=ost[64 * u : 64 * u + 32, :],
                )
                nc.gpsimd.dma_start(out=dn8[h : h + 1, :],
                                    in_=ost[64 * u + 32 : 64 * u + 33, :])

        for r in range(16):
            hp, jp = divmod(r, 4)
            heads = (2 * hp, 2 * hp + 1)
            use_inj = r in INJ_ROUNDS
            if use_inj:
                ebt = ebuf.tile([P, 2, 2 * NI], F8, tag="ebt", name=f"ebt{r}")
                nc.sync.dma_start(out=ebt, in_=eb_d[hp, jp])
            else:
                ebx = {}
                for u, h in enumerate(heads):
                    ebx[h] = ebuf.tile([P, 2, NI], BF16, tag="ebx",
                                       name=f"ebx{r}_{u}")
                    nc.sync.dma_start(out=ebx[h], in_=ebx_d[hp, jp, u])
            s_ps = {h: sslot() for h in heads}
            # QK batch: self-contained groups so the two heads' matmuls
            # co-execute in the array (different 32-row tile groups)
            qf = ql = None
            for t in range(2):
                jc = jp * 2 + t
                for h in heads:
                    hq, hc = h % 4, h // 4
                    ql = nc.tensor.matmul(
                        s_ps[h][:, ts(t, 512)],
                        lhsT=kt_sb[hc][hq * 32 : (hq + 1) * 32, ts(jc, P)],
                        rhs=qt_sb[hc][hq * 32 : (hq + 1) * 32, :],
                        start=True, stop=True,
                        tile_position=(hq * 32, 0),
                        skip_group_check=True,
                    )
                    qf = qf or ql
            last = ql
            if use_inj:
                # bias inject: accumulates B^T on top of each t-subregion
                # (matmul out can't cross a PSUM bank boundary)
                for u, h in enumerate(heads):
                    for t in range(2):
                        last = nc.tensor.matmul(
                            s_ps[h][:, ts(t, 512)],
                            lhsT=id_sb[:, :, ts(u, P)],
                            rhs=ebt[:, :, ts(t, 512)],
                            start=False, stop=True,
                            perf_mode=DROW,
                            skip_group_check=True,
                        )
                        add_dep_helper(last.ins, ql.ins, sync=False,
                                       reason="bias inject after qk batch")
            qk_insts.append((qf, last))
            # injected projection work between the QK and PV batches
            ij = []
            for fn in inject.get(r, []):
                ij.append(fn())
            inj_insts.append(ij)
            # exp per head: PSUM -> bf16 P^T in SBUF (multiply rounds apply
            # exp(bias) on the DVE afterwards)
            pt = {}
            for h in heads:
                pt[h] = ptb.tile([P, 2, NI], BF16, tag="pt", name=f"pt{h}_{jp}")
                if use_inj:
                    nc.scalar.activation(
                        out=pt[h],
                        in_=s_ps[h][:].rearrange("p (t i) -> p t i", t=2),
                        func=EXPF,
                    )
                else:
                    es = esb.tile([P, 2, NI], BF16, tag="es",
                                  name=f"es{h}_{jp}")
                    nc.scalar.activation(
                        out=es,
                        in_=s_ps[h][:].rearrange("p (t i) -> p t i", t=2),
                        func=EXPF,
                    )
                    nc.vector.tensor_mul(out=pt[h], in0=es, in1=ebx[h])
            pt_tiles[r] = pt
            if r >= 1:
                pv_insts.append(emit_pv(r - 1))
            if jp == 0 and hp >= 1:
                emit_dumps(hp - 1)
            if r == 9:
                chunk_tail(0, nc.gpsimd)
        pv_insts.append(emit_pv(15))
        emit_dumps(3)

        # PE batch order: [QK+inj][proj][PV] per round, staggered
        for r in range(len(qk_insts)):
            if r >= 2:
                add_dep_helper(qk_insts[r][0].ins, pv_insts[r - 2][1].ins,
                               sync=False, reason="qk(r) after pv(r-2)")
            for f, l in inj_insts[r]:
                add_dep_helper(f.ins, qk_insts[r][1].ins, sync=False,
                               reason="proj after qk batch")
            if r + 1 < len(qk_insts):
                prev = (inj_insts[r + 1][-1][1] if inj_insts[r + 1]
                        else qk_insts[r + 1][1])
                add_dep_helper(pv_insts[r][0].ins, prev.ins, sync=False,
                               reason="pv(r) after qk/inj(r+1)")

        chunk_tail(1, None)

        # ---- tail: output projection y^T[c, i] ----
        for cc in range(2):
            yps = sslot()
            for ec in range(2):
                nc.tensor.matmul(
                    yps[:, :NI], lhsT=wo_sb[:, ec, ts(cc, P)], rhs=ogt[:, ec, :],
                    start=(ec == 0), stop=(ec == 1),
                )
            ysb = ostb.tile([P, NI], F32, tag="ysb")
            nc.vector.tensor_scalar_add(out=ysb, in0=yps[:, :NI],
                                        scalar1=bob_sb[:, cc : cc + 1])
            nc.sync.dma_start(out=y_d[cc], in_=ysb)

    nc.compile()
    return nc


def prep_core_inputs(core, x, mask, attn_bias, Wq, Wkv, Wo, bo, Wg, bg):
    """Host-side shard + layout prep for one core. All numpy."""
    b, ih = core // 2, core % 2
    i0 = ih * NI
    scale = D ** -0.5

    xt = np.ascontiguousarray(x[b].T)  # [256, N]
    amask = np.where(mask[b] > 0, 0.0, -200.0).astype(np.float32)  # [N] over j
    bt = attn_bias[b, :, i0 : i0 + NI, :].transpose(0, 2, 1)  # [H, j, i]
    bt = bt + amask[None, :, None]
    bt6 = bt.reshape(4, 2, 4, 2, P, NI)  # [hp, u, jp, t, p, i]
    # inject rounds: raw fp8, paired:
    # eb[hp, jp, p, kt, t*512+i] = bt[2*hp+kt, (jp*2+t)*128+p, i]
    eb = (
        bt6.transpose(0, 2, 4, 1, 3, 5).reshape(4, 4, P, 2, 2 * NI)
    ).astype(NPF8)
    # multiply rounds: exp(bias) bf16 per head:
    ebx = np.exp(bt6.transpose(0, 2, 1, 4, 3, 5)).astype(NPBF16)  # [hp,jp,u,p,t,i]

    def chunk(wT):  # [256, X] -> [2, 128, X] bf16
        return np.ascontiguousarray(wT.reshape(2, P, -1)).astype(NPBF16)

    ind = np.zeros((8, 256), np.float32)
    for h in range(H):
        ind[h, h * 32 : (h + 1) * 32] = 1.0

    idm = np.zeros((P, 2, 2 * P), np.float32)
    for u in range(2):
        idm[:, u, u * P : (u + 1) * P] = np.eye(P)

    def flat(wT):  # [256, X] -> [128, 2*X] per-partition pack
        c = chunk(wT)  # [2, 128, X]
        return c.transpose(1, 0, 2).reshape(P, -1)

    wpack = np.concatenate(
        [flat((Wq * scale).T), flat(xt[:, i0 : i0 + NI]), flat(Wkv[:256].T),
         flat(xt), flat(Wkv[256:].T), flat(Wg.T), flat(Wo.T)], axis=1)
    return {
        "wpack": np.ascontiguousarray(wpack),
        "eb": np.ascontiguousarray(eb),
        "ebx": np.ascontiguousarray(ebx),
        "idm": idm.astype(NPF8),
        "hbg": np.ascontiguousarray((bg * 0.5).reshape(2, P).T).astype(np.float32),
        "bob": np.ascontiguousarray(bo.astype(np.float32).reshape(2, P).T),
        "ind": ind.astype(NPBF16),
    }


def prep_all_inputs(**inputs):
    inputs = {k: np.asarray(v, dtype=np.float32) for k, v in inputs.items()}
    return [prep_core_inputs(c, **inputs) for c in range(NCORES)]


def gather_outputs(results):
    """results: per-core dicts with 'out' = y^T chunked [2, P, NI] -> [B, N, DQ]."""
    y = np.zeros((B, N, DQ), np.float32)
    for c in range(NCORES):
        b, ih = c // 2, c % 2
        yt = np.asarray(results[c]["out"]).reshape(DQ, NI)  # [c, i]
        y[b, ih * NI : (ih + 1) * NI, :] = yt.T
    return y


_NC_CACHE = None


def _get_nc():
    global _NC_CACHE
    if _NC_CACHE is None:
        _NC_CACHE = build_nc()
    return _NC_CACHE


def kernel(**inputs):
    """Full (unsharded) inputs -> full [B, N, DQ] output, on 8 NeuronCores."""
    from concourse.bass_utils import run_bass_kernel_spmd

    nc = _get_nc()
    in_maps = prep_all_inputs(**inputs)
    res = run_bass_kernel_spmd(nc, in_maps, list(range(NCORES)))
    return gather_outputs(res.results)
